# revision 11
# baseline (speedup 1.0000x reference)
"""Trainium2 Bass kernel for nn_MinimalQuantumCell.

Computes, for full inputs
    x         [4096, 256]  f32
    h_quantum [4096, 512, 16] f32
    W_quantum [256, 512, 16]  f32
the pair
    output    [4096, 512]      = mean_s tanh(x @ W + 0.9 h)
    new_state [4096, 512, 16]  = tanh(x @ W + 0.9 h)

Strategy: model-parallel over the hid axis across 8 NeuronCores (64 hid
units -> 1024 (hid,s) columns each); x replicated (pre-transposed on
host), W sharded.  All per-core DRAM blocks are contiguous, so every
DMA is a simple linear transfer.  Per core:
  - x^T (4 MB) and the W shard (1 MB) are preloaded to SBUF.
  - For each [128 batch x 1024 col] tile: PSUM accumulates x@W over two
    K=128 matmuls (float32r - full-rate fp32 on the PE); VectorE adds
    0.9*h (pre-scaled on host) in exact fp32; ScalarE applies tanh;
    VectorE reduces groups of 16 states for the mean; DMA streams h
    in (sync queue) / state out (scalar queue).
"""

import numpy as np
from contextlib import ExitStack

BATCH, IN_DIM, HID, STATES = 4096, 256, 512, 16
N_CORES = 8
P = 128          # SBUF partitions
N_TILE = 512     # matmul moving-dim tile (one PSUM bank of f32)

_CACHE = {}

MM_DTYPE = "float32"   # "float32r" (fast, ~1e-3 err) or "float32" (exact)


def _build_nc(mm_dtype=MM_DTYPE, n_cores=N_CORES):
    import concourse.tile as tile
    from concourse import bacc, mybir

    N_LOC = HID * STATES // n_cores   # 1024 (hid,s) columns per core
    KT = IN_DIM // P                  # 2 k-tiles
    MT = BATCH // P                   # 32 m-tiles
    NT = N_LOC // N_TILE              # 2 n-tiles
    GP = N_TILE // STATES             # 32 hid groups per n-tile
    HID_LOC = HID // n_cores          # 64 hid units per core

    f32 = mybir.dt.float32
    mmdt = getattr(mybir.dt, mm_dtype)

    def r(ap):
        # view f32 DRAM data as the matmul dtype (pure reinterpret; the
        # PE rounds internally for f32r)
        return ap if mm_dtype == "float32" else ap.bitcast(mmdt)

    nc = bacc.Bacc(
        "TRN2", target_bir_lowering=False, debug=False, num_devices=n_cores
    )
    xT = nc.dram_tensor("xT", [IN_DIM, BATCH], f32, kind="ExternalInput").ap()
    w = nc.dram_tensor("w", [IN_DIM, N_LOC], f32, kind="ExternalInput").ap()
    h = nc.dram_tensor("h", [BATCH, N_LOC], f32, kind="ExternalInput").ap()
    state = nc.dram_tensor("state", [BATCH, N_LOC], f32, kind="ExternalOutput").ap()
    mean = nc.dram_tensor("mean", [BATCH, HID_LOC], f32, kind="ExternalOutput").ap()

    with tile.TileContext(nc) as tc, ExitStack() as ctx:
        singles = ctx.enter_context(tc.tile_pool(name="singles", bufs=1))
        hpool = ctx.enter_context(tc.tile_pool(name="hin", bufs=8))
        spool = ctx.enter_context(tc.tile_pool(name="stout", bufs=8))
        mpool = ctx.enter_context(tc.tile_pool(name="meanacc", bufs=4))
        pspool = ctx.enter_context(tc.tile_pool(name="psum", bufs=4, space="PSUM"))

        w_sb = singles.tile([P, KT, N_LOC], mmdt)
        for k in range(KT):
            nc.gpsimd.dma_start(w_sb[:, k, :], r(w[k * P:(k + 1) * P, :]))
        xT_sb = singles.tile([P, KT, BATCH], mmdt)
        # split the 2MB-per-k transfer into chunks, k-interleaved, so the
        # first m-tiles can start as soon as both k chunks of their batch
        # range have landed
        XC = 8
        for c in range(XC):
            csl = slice(c * (BATCH // XC), (c + 1) * (BATCH // XC))
            for k in range(KT):
                nc.gpsimd.dma_start(xT_sb[:, k, csl], r(xT[k * P:(k + 1) * P, csl]))

        for m in range(MT):
            msl = slice(m * P, (m + 1) * P)
            h_t = hpool.tile([P, N_LOC], f32)
            nc.sync.dma_start(h_t[:], h[msl, :])

            macc = mpool.tile([P, HID_LOC], f32)
            ps = pspool.tile([P, N_LOC], f32)   # 2 PSUM banks
            for k in range(KT):
                for n in range(NT):
                    nsl = slice(n * N_TILE, (n + 1) * N_TILE)
                    nc.tensor.matmul(
                        ps[:, nsl],
                        xT_sb[:, k, msl],
                        w_sb[:, k, nsl],
                        start=(k == 0),
                        stop=(k == KT - 1),
                    )
            st = spool.tile([P, N_LOC], f32)
            # exact fp32: st = psum + 0.9*h   (h pre-scaled on host)
            nc.vector.tensor_tensor(
                st[:], ps[:], h_t[:], op=mybir.AluOpType.add
            )
            nc.scalar.activation(
                st[:], st[:], mybir.ActivationFunctionType.Tanh
            )
            # sum over the 16 states per hid unit (scaled to mean on host)
            nc.vector.tensor_reduce(
                macc[:],
                st[:].rearrange("p (g s) -> p g s", s=STATES),
                axis=mybir.AxisListType.X,
                op=mybir.AluOpType.add,
            )
            nc.scalar.dma_start(state[msl, :], st[:])
            nc.gpsimd.dma_start(mean[msl, :], macc[:])

    nc.compile()
    return nc


def _get_nc():
    if "nc" not in _CACHE:
        _CACHE["nc"] = _build_nc()
    return _CACHE["nc"]


def _shard_inputs(x, h, W):
    N_LOC = HID * STATES // N_CORES
    xTf = np.ascontiguousarray(x.T)                       # [256, 4096]
    w2 = W.reshape(IN_DIM, HID * STATES)
    h2 = h.reshape(BATCH, HID * STATES)
    in_maps = []
    for c in range(N_CORES):
        sl = slice(c * N_LOC, (c + 1) * N_LOC)
        in_maps.append({
            "xT": xTf,
            "w": np.ascontiguousarray(w2[:, sl]),
            "h": h2[:, sl] * np.float32(0.9),
        })
    return in_maps


def kernel(x, h_quantum, W_quantum, _nc=None, _run_kwargs=None):
    from concourse.bass_utils import run_bass_kernel_spmd

    x = np.asarray(x, dtype=np.float32)
    h = np.asarray(h_quantum, dtype=np.float32)
    W = np.asarray(W_quantum, dtype=np.float32)

    nc = _nc if _nc is not None else _get_nc()
    in_maps = _shard_inputs(x, h, W)
    res = run_bass_kernel_spmd(
        nc, in_maps, core_ids=list(range(N_CORES)), **(_run_kwargs or {})
    )
    outs = res.results
    state = np.concatenate(
        [outs[c]["state"] for c in range(N_CORES)], axis=1
    ).reshape(BATCH, HID, STATES)
    mean = np.concatenate(
        [outs[c]["mean"] for c in range(N_CORES)], axis=1
    ) * np.float32(1.0 / STATES)
    if _run_kwargs:
        _CACHE["last_results"] = res
    return mean.astype(np.float32, copy=False), state.astype(np.float32, copy=False)


# revision 13
# speedup vs baseline: 1.0397x; 1.0397x over previous
"""Trainium2 Bass kernel for nn_MinimalQuantumCell.

Computes, for full inputs
    x         [4096, 256]  f32
    h_quantum [4096, 512, 16] f32
    W_quantum [256, 512, 16]  f32
the pair
    output    [4096, 512]      = mean_s tanh(x @ W + 0.9 h)
    new_state [4096, 512, 16]  = tanh(x @ W + 0.9 h)

Strategy: model-parallel over the hid axis across 8 NeuronCores (64 hid
units -> 1024 (hid,s) columns each); x replicated (pre-transposed on
host), W sharded.  All per-core DRAM blocks are contiguous, so every
DMA is a simple linear transfer.  Per core:
  - x^T (4 MB) and the W shard (1 MB) are preloaded to SBUF.
  - For each [128 batch x 1024 col] tile: PSUM accumulates x@W over two
    K=128 matmuls (float32r - full-rate fp32 on the PE); VectorE adds
    0.9*h (pre-scaled on host) in exact fp32; ScalarE applies tanh;
    VectorE reduces groups of 16 states for the mean; DMA streams h
    in (sync queue) / state out (scalar queue).
"""

import numpy as np
from contextlib import ExitStack

BATCH, IN_DIM, HID, STATES = 4096, 256, 512, 16
N_CORES = 8
P = 128          # SBUF partitions
N_TILE = 512     # matmul moving-dim tile (one PSUM bank of f32)

_CACHE = {}

MM_DTYPE = "float32r"   # "float32r" (fast, ~1e-3 err) or "float32" (exact)


def _build_nc(mm_dtype=MM_DTYPE, n_cores=N_CORES):
    import concourse.tile as tile
    from concourse import bacc, mybir

    N_LOC = HID * STATES // n_cores   # 1024 (hid,s) columns per core
    KT = IN_DIM // P                  # 2 k-tiles
    MT = BATCH // P                   # 32 m-tiles
    NT = N_LOC // N_TILE              # 2 n-tiles
    GP = N_TILE // STATES             # 32 hid groups per n-tile
    HID_LOC = HID // n_cores          # 64 hid units per core

    f32 = mybir.dt.float32
    mmdt = getattr(mybir.dt, mm_dtype)

    def r(ap):
        # view f32 DRAM data as the matmul dtype (pure reinterpret; the
        # PE rounds internally for f32r)
        return ap if mm_dtype == "float32" else ap.bitcast(mmdt)

    nc = bacc.Bacc(
        "TRN2", target_bir_lowering=False, debug=False, num_devices=n_cores
    )
    xT = nc.dram_tensor("xT", [IN_DIM, BATCH], f32, kind="ExternalInput").ap()
    w = nc.dram_tensor("w", [IN_DIM, N_LOC], f32, kind="ExternalInput").ap()
    h = nc.dram_tensor("h", [BATCH, N_LOC], f32, kind="ExternalInput").ap()
    state = nc.dram_tensor("state", [BATCH, N_LOC], f32, kind="ExternalOutput").ap()
    mean = nc.dram_tensor("mean", [BATCH, HID_LOC], f32, kind="ExternalOutput").ap()

    with tile.TileContext(nc) as tc, ExitStack() as ctx:
        singles = ctx.enter_context(tc.tile_pool(name="singles", bufs=1))
        hpool = ctx.enter_context(tc.tile_pool(name="hin", bufs=8))
        spool = ctx.enter_context(tc.tile_pool(name="stout", bufs=8))
        mpool = ctx.enter_context(tc.tile_pool(name="meanacc", bufs=4))
        pspool = ctx.enter_context(tc.tile_pool(name="psum", bufs=4, space="PSUM"))

        w_sb = singles.tile([P, KT, N_LOC], mmdt)
        for k in range(KT):
            nc.scalar.dma_start(w_sb[:, k, :], r(w[k * P:(k + 1) * P, :]))
        xT_sb = singles.tile([P, KT, BATCH], mmdt)
        # split the 2MB-per-k transfer into chunks, k-interleaved, so the
        # first m-tiles can start as soon as both k chunks of their batch
        # range have landed
        XC = 8
        for c in range(XC):
            csl = slice(c * (BATCH // XC), (c + 1) * (BATCH // XC))
            for k in range(KT):
                nc.scalar.dma_start(xT_sb[:, k, csl], r(xT[k * P:(k + 1) * P, csl]))

        for m in range(MT):
            msl = slice(m * P, (m + 1) * P)
            h_t = hpool.tile([P, N_LOC], f32)
            nc.sync.dma_start(h_t[:], h[msl, :])

            macc = mpool.tile([P, HID_LOC], f32)
            ps = pspool.tile([P, N_LOC], f32)   # 2 PSUM banks
            for k in range(KT):
                for n in range(NT):
                    nsl = slice(n * N_TILE, (n + 1) * N_TILE)
                    nc.tensor.matmul(
                        ps[:, nsl],
                        xT_sb[:, k, msl],
                        w_sb[:, k, nsl],
                        start=(k == 0),
                        stop=(k == KT - 1),
                    )
            st = spool.tile([P, N_LOC], f32)
            # exact fp32: st = psum + 0.9*h   (h pre-scaled on host)
            nc.vector.tensor_tensor(
                st[:], ps[:], h_t[:], op=mybir.AluOpType.add
            )
            nc.scalar.activation(
                st[:], st[:], mybir.ActivationFunctionType.Tanh
            )
            # sum over the 16 states per hid unit (scaled to mean on host)
            nc.vector.tensor_reduce(
                macc[:],
                st[:].rearrange("p (g s) -> p g s", s=STATES),
                axis=mybir.AxisListType.X,
                op=mybir.AluOpType.add,
            )
            nc.scalar.dma_start(state[msl, :], st[:])
            nc.gpsimd.dma_start(mean[msl, :], macc[:])

    nc.compile()
    return nc


def _get_nc():
    if "nc" not in _CACHE:
        _CACHE["nc"] = _build_nc()
    return _CACHE["nc"]


def _shard_inputs(x, h, W):
    N_LOC = HID * STATES // N_CORES
    xTf = np.ascontiguousarray(x.T)                       # [256, 4096]
    w2 = W.reshape(IN_DIM, HID * STATES)
    h2 = h.reshape(BATCH, HID * STATES)
    in_maps = []
    for c in range(N_CORES):
        sl = slice(c * N_LOC, (c + 1) * N_LOC)
        in_maps.append({
            "xT": xTf,
            "w": np.ascontiguousarray(w2[:, sl]),
            "h": h2[:, sl] * np.float32(0.9),
        })
    return in_maps


def kernel(x, h_quantum, W_quantum, _nc=None, _run_kwargs=None):
    from concourse.bass_utils import run_bass_kernel_spmd

    x = np.asarray(x, dtype=np.float32)
    h = np.asarray(h_quantum, dtype=np.float32)
    W = np.asarray(W_quantum, dtype=np.float32)

    nc = _nc if _nc is not None else _get_nc()
    in_maps = _shard_inputs(x, h, W)
    res = run_bass_kernel_spmd(
        nc, in_maps, core_ids=list(range(N_CORES)), **(_run_kwargs or {})
    )
    outs = res.results
    state = np.concatenate(
        [outs[c]["state"] for c in range(N_CORES)], axis=1
    ).reshape(BATCH, HID, STATES)
    mean = np.concatenate(
        [outs[c]["mean"] for c in range(N_CORES)], axis=1
    ) * np.float32(1.0 / STATES)
    if _run_kwargs:
        _CACHE["last_results"] = res
    return mean.astype(np.float32, copy=False), state.astype(np.float32, copy=False)


# revision 14
# speedup vs baseline: 1.0634x; 1.0228x over previous
"""Trainium2 Bass kernel for nn_MinimalQuantumCell.

Computes, for full inputs
    x         [4096, 256]  f32
    h_quantum [4096, 512, 16] f32
    W_quantum [256, 512, 16]  f32
the pair
    output    [4096, 512]      = mean_s tanh(x @ W + 0.9 h)
    new_state [4096, 512, 16]  = tanh(x @ W + 0.9 h)

Strategy: model-parallel over the hid axis across 8 NeuronCores (64 hid
units -> 1024 (hid,s) columns each); x replicated (pre-transposed on
host), W sharded.  All per-core DRAM blocks are contiguous, so every
DMA is a simple linear transfer.  Per core:
  - x^T (4 MB) and the W shard (1 MB) are preloaded to SBUF.
  - For each [128 batch x 1024 col] tile: PSUM accumulates x@W over two
    K=128 matmuls (float32r - full-rate fp32 on the PE); VectorE adds
    0.9*h (pre-scaled on host) in exact fp32; ScalarE applies tanh;
    VectorE reduces groups of 16 states for the mean; DMA streams h
    in (sync queue) / state out (scalar queue).
"""

import numpy as np
from contextlib import ExitStack

BATCH, IN_DIM, HID, STATES = 4096, 256, 512, 16
N_CORES = 8
P = 128          # SBUF partitions
N_TILE = 512     # matmul moving-dim tile (one PSUM bank of f32)

_CACHE = {}

MM_DTYPE = "float32r"   # "float32r" (fast, ~1e-3 err) or "float32" (exact)


def _build_nc(mm_dtype=MM_DTYPE, n_cores=N_CORES):
    import concourse.tile as tile
    from concourse import bacc, mybir

    N_LOC = HID * STATES // n_cores   # 1024 (hid,s) columns per core
    KT = IN_DIM // P                  # 2 k-tiles
    MT = BATCH // P                   # 32 m-tiles
    NT = N_LOC // N_TILE              # 2 n-tiles
    GP = N_TILE // STATES             # 32 hid groups per n-tile
    HID_LOC = HID // n_cores          # 64 hid units per core

    f32 = mybir.dt.float32
    mmdt = getattr(mybir.dt, mm_dtype)

    def r(ap):
        # view f32 DRAM data as the matmul dtype (pure reinterpret; the
        # PE rounds internally for f32r)
        return ap if mm_dtype == "float32" else ap.bitcast(mmdt)

    nc = bacc.Bacc(
        "TRN2", target_bir_lowering=False, debug=False, num_devices=n_cores
    )
    xT = nc.dram_tensor("xT", [IN_DIM, BATCH], f32, kind="ExternalInput").ap()
    w = nc.dram_tensor("w", [IN_DIM, N_LOC], f32, kind="ExternalInput").ap()
    h = nc.dram_tensor("h", [BATCH, N_LOC], f32, kind="ExternalInput").ap()
    state = nc.dram_tensor("state", [BATCH, N_LOC], f32, kind="ExternalOutput").ap()
    mean = nc.dram_tensor("mean", [BATCH, HID_LOC], f32, kind="ExternalOutput").ap()

    with tile.TileContext(nc) as tc, ExitStack() as ctx:
        singles = ctx.enter_context(tc.tile_pool(name="singles", bufs=1))
        hpool = ctx.enter_context(tc.tile_pool(name="hin", bufs=16))
        spool = ctx.enter_context(tc.tile_pool(name="stout", bufs=12))
        mpool = ctx.enter_context(tc.tile_pool(name="meanacc", bufs=4))
        pspool = ctx.enter_context(tc.tile_pool(name="psum", bufs=4, space="PSUM"))

        w_sb = singles.tile([P, KT, N_LOC], mmdt)
        for k in range(KT):
            nc.scalar.dma_start(w_sb[:, k, :], r(w[k * P:(k + 1) * P, :]))
        xT_sb = singles.tile([P, KT, BATCH], mmdt)
        # split the 2MB-per-k transfer into chunks, k-interleaved, so the
        # first m-tiles can start as soon as both k chunks of their batch
        # range have landed
        XC = 8
        for c in range(XC):
            csl = slice(c * (BATCH // XC), (c + 1) * (BATCH // XC))
            for k in range(KT):
                nc.scalar.dma_start(xT_sb[:, k, csl], r(xT[k * P:(k + 1) * P, csl]))

        for m in range(MT):
            msl = slice(m * P, (m + 1) * P)
            h_t = hpool.tile([P, N_LOC], f32)
            nc.sync.dma_start(h_t[:], h[msl, :])

            macc = mpool.tile([P, HID_LOC], f32)
            ps = pspool.tile([P, N_LOC], f32)   # 2 PSUM banks
            for k in range(KT):
                for n in range(NT):
                    nsl = slice(n * N_TILE, (n + 1) * N_TILE)
                    nc.tensor.matmul(
                        ps[:, nsl],
                        xT_sb[:, k, msl],
                        w_sb[:, k, nsl],
                        start=(k == 0),
                        stop=(k == KT - 1),
                    )
            st = spool.tile([P, N_LOC], f32)
            # exact fp32: st = psum + 0.9*h   (h pre-scaled on host)
            nc.vector.tensor_tensor(
                st[:], ps[:], h_t[:], op=mybir.AluOpType.add
            )
            nc.scalar.activation(
                st[:], st[:], mybir.ActivationFunctionType.Tanh
            )
            # sum over the 16 states per hid unit (scaled to mean on host)
            nc.vector.tensor_reduce(
                macc[:],
                st[:].rearrange("p (g s) -> p g s", s=STATES),
                axis=mybir.AxisListType.X,
                op=mybir.AluOpType.add,
            )
            nc.scalar.dma_start(state[msl, :], st[:])
            nc.gpsimd.dma_start(mean[msl, :], macc[:])

    nc.compile()
    return nc


def _get_nc():
    if "nc" not in _CACHE:
        _CACHE["nc"] = _build_nc()
    return _CACHE["nc"]


def _shard_inputs(x, h, W):
    N_LOC = HID * STATES // N_CORES
    xTf = np.ascontiguousarray(x.T)                       # [256, 4096]
    w2 = W.reshape(IN_DIM, HID * STATES)
    h2 = h.reshape(BATCH, HID * STATES)
    in_maps = []
    for c in range(N_CORES):
        sl = slice(c * N_LOC, (c + 1) * N_LOC)
        in_maps.append({
            "xT": xTf,
            "w": np.ascontiguousarray(w2[:, sl]),
            "h": h2[:, sl] * np.float32(0.9),
        })
    return in_maps


def kernel(x, h_quantum, W_quantum, _nc=None, _run_kwargs=None):
    from concourse.bass_utils import run_bass_kernel_spmd

    x = np.asarray(x, dtype=np.float32)
    h = np.asarray(h_quantum, dtype=np.float32)
    W = np.asarray(W_quantum, dtype=np.float32)

    nc = _nc if _nc is not None else _get_nc()
    in_maps = _shard_inputs(x, h, W)
    res = run_bass_kernel_spmd(
        nc, in_maps, core_ids=list(range(N_CORES)), **(_run_kwargs or {})
    )
    outs = res.results
    state = np.concatenate(
        [outs[c]["state"] for c in range(N_CORES)], axis=1
    ).reshape(BATCH, HID, STATES)
    mean = np.concatenate(
        [outs[c]["mean"] for c in range(N_CORES)], axis=1
    ) * np.float32(1.0 / STATES)
    if _run_kwargs:
        _CACHE["last_results"] = res
    return mean.astype(np.float32, copy=False), state.astype(np.float32, copy=False)


# revision 15
# speedup vs baseline: 1.0998x; 1.0342x over previous
"""Trainium2 Bass kernel for nn_MinimalQuantumCell.

Computes, for full inputs
    x         [4096, 256]  f32
    h_quantum [4096, 512, 16] f32
    W_quantum [256, 512, 16]  f32
the pair
    output    [4096, 512]      = mean_s tanh(x @ W + 0.9 h)
    new_state [4096, 512, 16]  = tanh(x @ W + 0.9 h)

Strategy: model-parallel over the hid axis across 8 NeuronCores (64 hid
units -> 1024 (hid,s) columns each); x replicated (pre-transposed on
host), W sharded.  All per-core DRAM blocks are contiguous, so every
DMA is a simple linear transfer.  Per core:
  - x^T and the W shard are preloaded to SBUF, each split on the host
    into bf16 hi/lo pairs (x = xh + xl exactly to ~2^-18 rel).
  - For each [128 batch x 1024 col] tile: PSUM accumulates
    xh@Wh + xh@Wl + xl@Wh over two K=128 tiles (error ~2e-5, at full
    bf16 PE rate); VectorE adds 0.9*h (pre-scaled on host) in exact
    fp32; ScalarE applies tanh; VectorE reduces groups of 16 states
    for the mean; DMA streams h in (sync queue) / state out (scalar
    queue).
"""

import numpy as np
from contextlib import ExitStack

BATCH, IN_DIM, HID, STATES = 4096, 256, 512, 16
N_CORES = 8
P = 128          # SBUF partitions
N_TILE = 512     # matmul moving-dim tile (one PSUM bank of f32)

_CACHE = {}

MM_MODE = "bf16x3"   # "bf16x3" | "float32r" | "float32"


def _build_nc(mm_mode=MM_MODE, n_cores=N_CORES):
    import concourse.tile as tile
    from concourse import bacc, mybir

    N_LOC = HID * STATES // n_cores   # 1024 (hid,s) columns per core
    KT = IN_DIM // P                  # 2 k-tiles
    MT = BATCH // P                   # 32 m-tiles
    NT = N_LOC // N_TILE              # 2 n-tiles
    HID_LOC = HID // n_cores          # 64 hid units per core

    f32 = mybir.dt.float32
    bf16 = mybir.dt.bfloat16
    split = mm_mode == "bf16x3"
    mmdt = bf16 if split else getattr(mybir.dt, mm_mode)

    def r(ap):
        return ap if mm_mode == "float32" or split else ap.bitcast(mmdt)

    nc = bacc.Bacc(
        "TRN2", target_bir_lowering=False, debug=False, num_devices=n_cores
    )
    xparts = ("xTh", "xTl") if split else ("xT",)
    wparts = ("wh", "wl") if split else ("w",)
    xdram = {
        nm: nc.dram_tensor(nm, [IN_DIM, BATCH], mmdt if split else f32,
                           kind="ExternalInput").ap()
        for nm in xparts
    }
    wdram = {
        nm: nc.dram_tensor(nm, [IN_DIM, N_LOC], mmdt if split else f32,
                           kind="ExternalInput").ap()
        for nm in wparts
    }
    h = nc.dram_tensor("h", [BATCH, N_LOC], f32, kind="ExternalInput").ap()
    state = nc.dram_tensor("state", [BATCH, N_LOC], f32, kind="ExternalOutput").ap()
    mean = nc.dram_tensor("mean", [BATCH, HID_LOC], f32, kind="ExternalOutput").ap()

    with tile.TileContext(nc) as tc, ExitStack() as ctx:
        singles = ctx.enter_context(tc.tile_pool(name="singles", bufs=1))
        hpool = ctx.enter_context(tc.tile_pool(name="hin", bufs=16))
        spool = ctx.enter_context(tc.tile_pool(name="stout", bufs=12))
        mpool = ctx.enter_context(tc.tile_pool(name="meanacc", bufs=4))
        pspool = ctx.enter_context(tc.tile_pool(name="psum", bufs=4, space="PSUM"))

        w_sb = {}
        for nm in wparts:
            w_sb[nm] = singles.tile([P, KT, N_LOC], mmdt, name=f"sb_{nm}",
                                    tag=f"sb_{nm}")
            for k in range(KT):
                nc.scalar.dma_start(w_sb[nm][:, k, :], r(wdram[nm][k * P:(k + 1) * P, :]))
        x_sb = {}
        XC = 4
        for nm in xparts:
            x_sb[nm] = singles.tile([P, KT, BATCH], mmdt, name=f"sb_{nm}",
                                    tag=f"sb_{nm}")
        # chunk the x preload, k- and part-interleaved, so early m-tiles
        # can start as soon as their batch range has landed
        for c in range(XC):
            csl = slice(c * (BATCH // XC), (c + 1) * (BATCH // XC))
            for k in range(KT):
                for nm in xparts:
                    nc.scalar.dma_start(
                        x_sb[nm][:, k, csl], r(xdram[nm][k * P:(k + 1) * P, csl])
                    )

        # matmul term schedule per m-tile: stationary operand grouped so
        # consecutive MMs share it; hi@hi+hi@lo+lo@hi for bf16x3
        if split:
            terms = []
            for k in range(KT):
                terms += [("xTh", k, "wh"), ("xTh", k, "wl")]
            for k in range(KT):
                terms += [("xTl", k, "wh")]
        else:
            terms = [(xparts[0], k, wparts[0]) for k in range(KT)]

        for m in range(MT):
            msl = slice(m * P, (m + 1) * P)
            h_t = hpool.tile([P, N_LOC], f32)
            nc.sync.dma_start(h_t[:], h[msl, :])

            macc = mpool.tile([P, HID_LOC], f32)
            ps = pspool.tile([P, N_LOC], f32)   # 2 PSUM banks
            for i, (xnm, k, wnm) in enumerate(terms):
                for n in range(NT):
                    nsl = slice(n * N_TILE, (n + 1) * N_TILE)
                    nc.tensor.matmul(
                        ps[:, nsl],
                        x_sb[xnm][:, k, msl],
                        w_sb[wnm][:, k, nsl],
                        start=(i == 0),
                        stop=(i == len(terms) - 1),
                    )
            st = spool.tile([P, N_LOC], f32)
            # exact fp32: st = psum + 0.9*h   (h pre-scaled on host)
            nc.vector.tensor_tensor(
                st[:], ps[:], h_t[:], op=mybir.AluOpType.add
            )
            nc.scalar.activation(
                st[:], st[:], mybir.ActivationFunctionType.Tanh
            )
            # sum over the 16 states per hid unit (scaled to mean on host)
            nc.vector.tensor_reduce(
                macc[:],
                st[:].rearrange("p (g s) -> p g s", s=STATES),
                axis=mybir.AxisListType.X,
                op=mybir.AluOpType.add,
            )
            nc.scalar.dma_start(state[msl, :], st[:])
            nc.gpsimd.dma_start(mean[msl, :], macc[:])

    nc.compile()
    return nc


def _get_nc():
    if "nc" not in _CACHE:
        _CACHE["nc"] = _build_nc()
    return _CACHE["nc"]


def _bf16_split(a):
    import ml_dtypes

    hi = a.astype(ml_dtypes.bfloat16)
    lo = (a - hi.astype(np.float32)).astype(ml_dtypes.bfloat16)
    return hi, lo


def _shard_inputs(x, h, W, mm_mode=MM_MODE):
    N_LOC = HID * STATES // N_CORES
    xTf = np.ascontiguousarray(x.T)                       # [256, 4096]
    w2 = W.reshape(IN_DIM, HID * STATES)
    h2 = h.reshape(BATCH, HID * STATES)
    if mm_mode == "bf16x3":
        xh, xl = _bf16_split(xTf)
        xin = {"xTh": xh, "xTl": xl}
    else:
        xin = {"xT": xTf}
    in_maps = []
    for c in range(N_CORES):
        sl = slice(c * N_LOC, (c + 1) * N_LOC)
        wsh = np.ascontiguousarray(w2[:, sl])
        if mm_mode == "bf16x3":
            wh, wl = _bf16_split(wsh)
            win = {"wh": wh, "wl": wl}
        else:
            win = {"w": wsh}
        in_maps.append({
            **xin, **win,
            "h": h2[:, sl] * np.float32(0.9),
        })
    return in_maps


def kernel(x, h_quantum, W_quantum, _nc=None, _run_kwargs=None):
    from concourse.bass_utils import run_bass_kernel_spmd

    x = np.asarray(x, dtype=np.float32)
    h = np.asarray(h_quantum, dtype=np.float32)
    W = np.asarray(W_quantum, dtype=np.float32)

    nc = _nc if _nc is not None else _get_nc()
    in_maps = _shard_inputs(x, h, W)
    res = run_bass_kernel_spmd(
        nc, in_maps, core_ids=list(range(N_CORES)), **(_run_kwargs or {})
    )
    outs = res.results
    state = np.concatenate(
        [outs[c]["state"] for c in range(N_CORES)], axis=1
    ).reshape(BATCH, HID, STATES)
    mean = np.concatenate(
        [outs[c]["mean"] for c in range(N_CORES)], axis=1
    ) * np.float32(1.0 / STATES)
    if _run_kwargs:
        _CACHE["last_results"] = res
    return mean.astype(np.float32, copy=False), state.astype(np.float32, copy=False)


# revision 16
# speedup vs baseline: 1.1080x; 1.0075x over previous
"""Trainium2 Bass kernel for nn_MinimalQuantumCell.

Computes, for full inputs
    x         [4096, 256]  f32
    h_quantum [4096, 512, 16] f32
    W_quantum [256, 512, 16]  f32
the pair
    output    [4096, 512]      = mean_s tanh(x @ W + 0.9 h)
    new_state [4096, 512, 16]  = tanh(x @ W + 0.9 h)

Strategy: model-parallel over the hid axis across 8 NeuronCores (64 hid
units -> 1024 (hid,s) columns each); x replicated (pre-transposed on
host), W sharded.  All per-core DRAM blocks are contiguous, so every
DMA is a simple linear transfer.  Per core:
  - x^T and the W shard are preloaded to SBUF, each split on the host
    into bf16 hi/lo pairs (x = xh + xl exactly to ~2^-18 rel).
  - For each [128 batch x 1024 col] tile: PSUM accumulates
    xh@Wh + xh@Wl + xl@Wh over two K=128 tiles (error ~2e-5, at full
    bf16 PE rate); VectorE adds 0.9*h (pre-scaled on host) in exact
    fp32; ScalarE applies tanh; VectorE reduces groups of 16 states
    for the mean; DMA streams h in (sync queue) / state out (scalar
    queue).
"""

import numpy as np
from contextlib import ExitStack

BATCH, IN_DIM, HID, STATES = 4096, 256, 512, 16
N_CORES = 8
P = 128          # SBUF partitions
N_TILE = 512     # matmul moving-dim tile (one PSUM bank of f32)

_CACHE = {}

MM_MODE = "bf16x3"   # "bf16x3" | "float32r" | "float32"


def _build_nc(mm_mode=MM_MODE, n_cores=N_CORES):
    import concourse.tile as tile
    from concourse import bacc, mybir

    N_LOC = HID * STATES // n_cores   # 1024 (hid,s) columns per core
    KT = IN_DIM // P                  # 2 k-tiles
    MT = BATCH // P                   # 32 m-tiles
    NT = N_LOC // N_TILE              # 2 n-tiles
    HID_LOC = HID // n_cores          # 64 hid units per core

    f32 = mybir.dt.float32
    bf16 = mybir.dt.bfloat16
    split = mm_mode == "bf16x3"
    mmdt = bf16 if split else getattr(mybir.dt, mm_mode)

    def r(ap):
        return ap if mm_mode == "float32" or split else ap.bitcast(mmdt)

    nc = bacc.Bacc(
        "TRN2", target_bir_lowering=False, debug=False, num_devices=n_cores
    )
    xparts = ("xTh", "xTl") if split else ("xT",)
    wparts = ("wh", "wl") if split else ("w",)
    xdram = {
        nm: nc.dram_tensor(nm, [IN_DIM, BATCH], mmdt if split else f32,
                           kind="ExternalInput").ap()
        for nm in xparts
    }
    wdram = {
        nm: nc.dram_tensor(nm, [IN_DIM, N_LOC], mmdt if split else f32,
                           kind="ExternalInput").ap()
        for nm in wparts
    }
    h = nc.dram_tensor("h", [BATCH, N_LOC], f32, kind="ExternalInput").ap()
    state = nc.dram_tensor("state", [BATCH, N_LOC], f32, kind="ExternalOutput").ap()
    mean = nc.dram_tensor("mean", [BATCH, HID_LOC], f32, kind="ExternalOutput").ap()

    with tile.TileContext(nc) as tc, ExitStack() as ctx:
        singles = ctx.enter_context(tc.tile_pool(name="singles", bufs=1))
        hpool = ctx.enter_context(tc.tile_pool(name="hin", bufs=16))
        spool = ctx.enter_context(tc.tile_pool(name="stout", bufs=12))
        mpool = ctx.enter_context(tc.tile_pool(name="meanacc", bufs=4))
        pspool = ctx.enter_context(tc.tile_pool(name="psum", bufs=4, space="PSUM"))

        w_sb = {}
        for nm in wparts:
            w_sb[nm] = singles.tile([P, KT, N_LOC], mmdt, name=f"sb_{nm}",
                                    tag=f"sb_{nm}")
        x_sb = {}
        XC = 4
        for nm in xparts:
            x_sb[nm] = singles.tile([P, KT, BATCH], mmdt, name=f"sb_{nm}",
                                    tag=f"sb_{nm}")
        # Preload order matters: the first m-tile's matmul deps go first
        # (wh k0, xh k0c0, ...), interleaved across both HWDGE queues so
        # issue rate doesn't serialize the head of the pipeline.
        preload = []
        for k in range(KT):
            preload.append((w_sb[wparts[0]][:, k, :],
                            r(wdram[wparts[0]][k * P:(k + 1) * P, :])))
            for nm in xparts:
                csl = slice(0, BATCH // XC)
                preload.append((x_sb[nm][:, k, csl],
                                r(xdram[nm][k * P:(k + 1) * P, csl])))
        if split:
            for k in range(KT):
                preload.append((w_sb["wl"][:, k, :],
                                r(wdram["wl"][k * P:(k + 1) * P, :])))
        for c in range(1, XC):
            csl = slice(c * (BATCH // XC), (c + 1) * (BATCH // XC))
            for k in range(KT):
                for nm in xparts:
                    preload.append((x_sb[nm][:, k, csl],
                                    r(xdram[nm][k * P:(k + 1) * P, csl])))
        for i, (dst, src) in enumerate(preload):
            eng = nc.scalar if i % 2 == 0 else nc.sync
            eng.dma_start(dst, src)

        # matmul term schedule per m-tile: stationary operand grouped so
        # consecutive MMs share it; hi@hi+hi@lo+lo@hi for bf16x3
        if split:
            terms = []
            for k in range(KT):
                terms += [("xTh", k, "wh"), ("xTh", k, "wl")]
            for k in range(KT):
                terms += [("xTl", k, "wh")]
        else:
            terms = [(xparts[0], k, wparts[0]) for k in range(KT)]

        for m in range(MT):
            msl = slice(m * P, (m + 1) * P)
            h_t = hpool.tile([P, N_LOC], f32)
            nc.sync.dma_start(h_t[:], h[msl, :])

            macc = mpool.tile([P, HID_LOC], f32)
            ps = pspool.tile([P, N_LOC], f32)   # 2 PSUM banks
            for i, (xnm, k, wnm) in enumerate(terms):
                for n in range(NT):
                    nsl = slice(n * N_TILE, (n + 1) * N_TILE)
                    nc.tensor.matmul(
                        ps[:, nsl],
                        x_sb[xnm][:, k, msl],
                        w_sb[wnm][:, k, nsl],
                        start=(i == 0),
                        stop=(i == len(terms) - 1),
                    )
            st = spool.tile([P, N_LOC], f32)
            # exact fp32: st = psum + 0.9*h   (h pre-scaled on host)
            nc.vector.tensor_tensor(
                st[:], ps[:], h_t[:], op=mybir.AluOpType.add
            )
            nc.scalar.activation(
                st[:], st[:], mybir.ActivationFunctionType.Tanh
            )
            # sum over the 16 states per hid unit (scaled to mean on host)
            nc.vector.tensor_reduce(
                macc[:],
                st[:].rearrange("p (g s) -> p g s", s=STATES),
                axis=mybir.AxisListType.X,
                op=mybir.AluOpType.add,
            )
            nc.scalar.dma_start(state[msl, :], st[:])
            nc.gpsimd.dma_start(mean[msl, :], macc[:])

    nc.compile()
    return nc


def _get_nc():
    if "nc" not in _CACHE:
        _CACHE["nc"] = _build_nc()
    return _CACHE["nc"]


def _bf16_split(a):
    import ml_dtypes

    hi = a.astype(ml_dtypes.bfloat16)
    lo = (a - hi.astype(np.float32)).astype(ml_dtypes.bfloat16)
    return hi, lo


def _shard_inputs(x, h, W, mm_mode=MM_MODE):
    N_LOC = HID * STATES // N_CORES
    xTf = np.ascontiguousarray(x.T)                       # [256, 4096]
    w2 = W.reshape(IN_DIM, HID * STATES)
    h2 = h.reshape(BATCH, HID * STATES)
    if mm_mode == "bf16x3":
        xh, xl = _bf16_split(xTf)
        xin = {"xTh": xh, "xTl": xl}
    else:
        xin = {"xT": xTf}
    in_maps = []
    for c in range(N_CORES):
        sl = slice(c * N_LOC, (c + 1) * N_LOC)
        wsh = np.ascontiguousarray(w2[:, sl])
        if mm_mode == "bf16x3":
            wh, wl = _bf16_split(wsh)
            win = {"wh": wh, "wl": wl}
        else:
            win = {"w": wsh}
        in_maps.append({
            **xin, **win,
            "h": h2[:, sl] * np.float32(0.9),
        })
    return in_maps


def kernel(x, h_quantum, W_quantum, _nc=None, _run_kwargs=None):
    from concourse.bass_utils import run_bass_kernel_spmd

    x = np.asarray(x, dtype=np.float32)
    h = np.asarray(h_quantum, dtype=np.float32)
    W = np.asarray(W_quantum, dtype=np.float32)

    nc = _nc if _nc is not None else _get_nc()
    in_maps = _shard_inputs(x, h, W)
    res = run_bass_kernel_spmd(
        nc, in_maps, core_ids=list(range(N_CORES)), **(_run_kwargs or {})
    )
    outs = res.results
    state = np.concatenate(
        [outs[c]["state"] for c in range(N_CORES)], axis=1
    ).reshape(BATCH, HID, STATES)
    mean = np.concatenate(
        [outs[c]["mean"] for c in range(N_CORES)], axis=1
    ) * np.float32(1.0 / STATES)
    if _run_kwargs:
        _CACHE["last_results"] = res
    return mean.astype(np.float32, copy=False), state.astype(np.float32, copy=False)
